# revision 4
# baseline (speedup 1.0000x reference)
"""Trainium2 Bass kernel for causal MHA (b=2, n=4096, d_model=768, 12 heads).

Sharding: 8 cores = 2 batches x 4 head-groups (3 heads each).
Each core:
  - receives its batch's Q/K/V pre-transposed ([768, n], d_model on rows)
    plus its head-group's weight slices (also pre-transposed on host).
  - projects qT/kT ([64, n] per head, head dim on partitions) and
    v ([n, 64] per head, tokens on partitions) on-chip.
  - computes scoresT[k, q] = kT^T @ qT tile-by-tile (128 keys x 512 queries),
    exponentiates (no max-subtraction: scores ~ N(0,1), fp32 exp is safe),
    masks the causal boundary with a precomputed 0/1 mask, and accumulates
    outT_aug[65, q] += [v | ones]^T @ P in PSUM.  Row 64 is the softmax
    denominator; division is folded into the PSUM->SBUF copy.
  - applies the output projection with its w_o row-slice; host sums the
    4 partial outputs per batch (row-parallel linear unshard).
"""

import sys

for _p in ("/opt/trn_rl_repo",):
    if _p not in sys.path:
        sys.path.insert(0, _p)

import numpy as np

import concourse.bass as bass  # noqa: F401  (registers engine classes)
import concourse.tile as tile
from concourse import bacc, mybir
import concourse.bass_utils as bass_utils

P = 128
D_MODEL = 768
KO = D_MODEL // P  # 6 contraction chunks of 128
N_HEADS = 12
D_K = 64
N_CORES = 8
H_LOCAL = 3  # heads per core
D_LOCAL = H_LOCAL * D_K  # 192
B = 2
N_TOKENS = 4096
NQ = 512  # query-chunk size (one PSUM bank of fp32)
NT = 256  # token chunk for q/k projection

F32 = mybir.dt.float32
BF16 = mybir.dt.bfloat16
F32R = mybir.dt.float32r


def _mm(ap, flavor):
    """View an fp32 AP as the matmul input dtype."""
    if flavor == "f32r":
        return ap.bitcast(F32R)
    return ap


def build_nc(n=N_TOKENS, mm="f32", dt_x=F32, dt_pt=F32, dt_acc=F32):
    """Build the single-core SPMD program.

    mm: "f32" | "f32r" | "bf16" - matmul input flavor. With "bf16",
        dt_x/dt_pt/dt_acc should be BF16 and inputs are cast on host.
    dt_x:  dtype of streamed X inputs and weights (DRAM + SBUF).
    dt_pt: dtype of exp'd probability tiles.
    dt_acc: dtype of on-chip qkT/v/outT activations.
    """
    assert n % NQ == 0 and n % NT == 0 and n % P == 0
    nc = bacc.Bacc("TRN2", target_bir_lowering=False, debug=False,
                   num_devices=N_CORES)

    qt_d = nc.dram_tensor("qt", [D_MODEL, n], dt_x, kind="ExternalInput")
    kt_d = nc.dram_tensor("kt", [D_MODEL, n], dt_x, kind="ExternalInput")
    vt_d = nc.dram_tensor("vt", [D_MODEL, n], dt_x, kind="ExternalInput")
    wqk_d = nc.dram_tensor("wqk", [D_MODEL, 2 * D_LOCAL], dt_x,
                           kind="ExternalInput")
    wv_d = nc.dram_tensor("wv", [D_MODEL, D_LOCAL], dt_x, kind="ExternalInput")
    wo_d = nc.dram_tensor("wo", [D_LOCAL, D_MODEL], dt_x, kind="ExternalInput")
    cm_d = nc.dram_tensor("cmask", [P, 4 * NQ], dt_pt, kind="ExternalInput")
    y_d = nc.dram_tensor("y", [n, D_MODEL], F32, kind="ExternalOutput")

    qt_r = qt_d.ap().rearrange("(ko ki) t -> ki ko t", ki=P)
    kt_r = kt_d.ap().rearrange("(ko ki) t -> ki ko t", ki=P)
    vt_r = vt_d.ap().rearrange("(ko ki) t -> ki ko t", ki=P)
    wqk_r = wqk_d.ap().rearrange("(ko ki) m -> ki ko m", ki=P)
    wv_r = wv_d.ap().rearrange("(ko ki) m -> ki ko m", ki=P)

    TCH = n // NT       # q/k projection token chunks
    TB = n // P         # 128-token blocks
    QCH = n // NQ       # query chunks
    KB_PER_Q = NQ // P  # key blocks per query chunk (4)

    # qkT_sb block layout (4 M-blocks):
    #   blk0 = [q_h0 (p 0-63) ; q_h1 (p 64-127)]
    #   blk1 = [q_h2 (p 0-63)]
    #   blk2 = [k_h0 ; k_h1],  blk3 = [k_h2]
    MBLOCKS = [(0, 128, "q", 0), (128, 64, "q", 1),
               (192, 128, "k", 2), (320, 64, "k", 3)]
    q_loc = {0: (0, 0), 1: (64, 0), 2: (0, 1)}
    k_loc = {0: (0, 2), 1: (64, 2), 2: (0, 3)}

    with tile.TileContext(nc) as tc:
        with tc.tile_pool(name="const", bufs=1) as cpool, \
             tc.tile_pool(name="persist", bufs=1) as ppool, \
             tc.tile_pool(name="xqk", bufs=3) as xpool, \
             tc.tile_pool(name="xv", bufs=2) as xvpool, \
             tc.tile_pool(name="pt", bufs=2) as ptpool, \
             tc.tile_pool(name="ysb", bufs=2) as ypool, \
             tc.tile_pool(name="rcp", bufs=2) as rpool, \
             tc.tile_pool(name="dbounce", bufs=2, space="DRAM") as dpool, \
             tc.tile_pool(name="pp_proj", bufs=2, space="PSUM") as pp_proj, \
             tc.tile_pool(name="pp_sc", bufs=3, space="PSUM") as pp_sc, \
             tc.tile_pool(name="pp_out", bufs=2, space="PSUM") as pp_out:

            # ---- constants ----
            wqk_sb = cpool.tile([P, KO, 2 * D_LOCAL], dt_x)
            nc.sync.dma_start(wqk_sb[:], wqk_r)
            wv_sb = cpool.tile([P, KO, D_LOCAL], dt_x)
            nc.sync.dma_start(wv_sb[:], wv_r)
            # w_o rows split per head: chunk h at partitions 0-63
            wo_sb = cpool.tile([64, H_LOCAL, D_MODEL], dt_x)
            for h in range(H_LOCAL):
                nc.sync.dma_start(wo_sb[:, h, :], wo_d.ap()[h * 64:(h + 1) * 64, :])
            cm_sb = cpool.tile([P, KB_PER_Q, NQ], dt_pt)
            nc.sync.dma_start(cm_sb[:], cm_d.ap().rearrange("p (k q) -> p k q", k=KB_PER_Q))

            # ---- persistent activations ----
            qkT_sb = ppool.tile([P, 4, n], dt_acc)
            v_sb = ppool.tile([P, TB, H_LOCAL, 66], dt_acc)
            outT_sb = ppool.tile([64, H_LOCAL, n], dt_acc)
            nc.vector.memset(v_sb[:, :, :, 64:65], 1.0)

            # ---- q/k projections (transposed layout) ----
            for t in range(TCH):
                xq = xpool.tile([P, KO, NT], dt_x, tag="x")
                nc.sync.dma_start(xq[:], qt_r[:, :, t * NT:(t + 1) * NT])
                xk = xpool.tile([P, KO, NT], dt_x, tag="x")
                nc.sync.dma_start(xk[:], kt_r[:, :, t * NT:(t + 1) * NT])
                for (coff, w, src, blk) in MBLOCKS:
                    x = xq if src == "q" else xk
                    ps = pp_proj.tile([P, NQ], F32, tag="psproj")
                    for ko in range(KO):
                        nc.tensor.matmul(
                            ps[0:w, 0:NT],
                            _mm(wqk_sb[:, ko, coff:coff + w], mm),
                            _mm(x[:, ko, :], mm),
                            start=(ko == 0), stop=(ko == KO - 1),
                        )
                    nc.vector.tensor_copy(
                        out=qkT_sb[0:w, blk, t * NT:(t + 1) * NT],
                        in_=ps[0:w, 0:NT],
                    )

            # ---- v projection (token-major layout) ----
            for tb in range(TB):
                xv = xvpool.tile([P, KO, P], dt_x)
                nc.sync.dma_start(xv[:], vt_r[:, :, tb * P:(tb + 1) * P])
                ps = pp_proj.tile([P, NQ], F32, tag="psproj")
                for ko in range(KO):
                    nc.tensor.matmul(
                        ps[:, 0:D_LOCAL],
                        _mm(xv[:, ko, :], mm),
                        _mm(wv_sb[:, ko, :], mm),
                        start=(ko == 0), stop=(ko == KO - 1),
                    )
                for h in range(H_LOCAL):
                    nc.vector.tensor_copy(
                        out=v_sb[:, tb, h, 0:64],
                        in_=ps[:, h * 64:(h + 1) * 64],
                    )

            # ---- causal attention, transposed-score flash style ----
            for h in range(H_LOCAL):
                qp, qb = q_loc[h]
                kp, kb_ = k_loc[h]
                for j in range(QCH):
                    po = pp_out.tile([P, NQ], F32, tag="po")
                    nkb = KB_PER_Q * j + KB_PER_Q
                    for kb in range(nkb):
                        psc = pp_sc.tile([P, NQ], F32, tag="psc")
                        nc.tensor.matmul(
                            psc[:],
                            _mm(qkT_sb[kp:kp + 64, kb_, kb * P:(kb + 1) * P], mm),
                            _mm(qkT_sb[qp:qp + 64, qb, j * NQ:(j + 1) * NQ], mm),
                            start=True, stop=True,
                        )
                        pt = ptpool.tile([P, NQ], dt_pt)
                        nc.scalar.activation(pt[:], psc[:],
                                             mybir.ActivationFunctionType.Exp)
                        kloc = kb - KB_PER_Q * j
                        if kloc >= 0:
                            nc.vector.tensor_mul(out=pt[:], in0=pt[:],
                                                 in1=cm_sb[:, kloc, :])
                        nc.tensor.matmul(
                            po[0:65, :],
                            _mm(v_sb[:, kb, h, 0:65], mm),
                            _mm(pt[:], mm),
                            start=(kb == 0), stop=(kb == nkb - 1),
                        )
                    # normalize: outT = po[0:64] * (1 / po[64])
                    r1 = rpool.tile([65, NQ], F32, tag="r1")
                    nc.vector.reciprocal(r1[64:65, :], po[64:65, :])
                    db = dpool.tile([1, NQ], F32)
                    nc.sync.dma_start(db[:], r1[64:65, :])
                    rr = rpool.tile([64, NQ], F32, tag="rr")
                    nc.sync.dma_start(rr[:], db[:].to_broadcast((64, NQ)))
                    nc.vector.tensor_mul(
                        out=outT_sb[:, h, j * NQ:(j + 1) * NQ],
                        in0=po[0:64, :], in1=rr[:],
                    )

            # ---- output projection ----
            NOC = 2  # 768 = 2 x 384
            NO = D_MODEL // NOC
            for tb in range(TB):
                for oc in range(NOC):
                    ps = pp_proj.tile([P, NQ], F32, tag="psproj")
                    for h in range(H_LOCAL):
                        nc.tensor.matmul(
                            ps[:, 0:NO],
                            _mm(outT_sb[:, h, tb * P:(tb + 1) * P], mm),
                            _mm(wo_sb[:, h, oc * NO:(oc + 1) * NO], mm),
                            start=(h == 0), stop=(h == H_LOCAL - 1),
                        )
                    ysb = ypool.tile([P, NO], F32)
                    nc.vector.tensor_copy(out=ysb[:], in_=ps[:, 0:NO])
                    nc.sync.dma_start(
                        y_d.ap()[tb * P:(tb + 1) * P, oc * NO:(oc + 1) * NO],
                        ysb[:],
                    )

    nc.compile()
    return nc


def make_causal_mask_np(dt=np.float32):
    """[128, 4*NQ]: mask[p, kloc*NQ + f] = 1.0 iff f >= p + kloc*128."""
    m = np.zeros((P, 4, NQ), dtype=np.float32)
    f = np.arange(NQ)[None, :]
    p = np.arange(P)[:, None]
    for kloc in range(4):
        m[:, kloc, :] = (f >= p + kloc * P).astype(np.float32)
    return m.reshape(P, 4 * NQ).astype(dt)


def prep_core_inputs(Q, K, V, w_q, w_k, w_v, w_o, core, n=N_TOKENS,
                     np_x=np.float32, np_pt=np.float32):
    """Host-side sharding/layout prep for one core. All fp32 numpy in."""
    b = core // 4
    g = core % 4
    hs = g * D_LOCAL
    scale = 1.0 / np.sqrt(D_K)
    qt = np.ascontiguousarray(Q[b].T).astype(np_x)
    kt = np.ascontiguousarray(K[b].T).astype(np_x)
    vt = np.ascontiguousarray(V[b].T).astype(np_x)
    wqk = np.ascontiguousarray(
        np.concatenate([w_q[hs:hs + D_LOCAL] * scale,
                        w_k[hs:hs + D_LOCAL]], axis=0).T).astype(np_x)
    wv = np.ascontiguousarray(w_v[hs:hs + D_LOCAL].T).astype(np_x)
    wo = np.ascontiguousarray(w_o[:, hs:hs + D_LOCAL].T).astype(np_x)
    cm = make_causal_mask_np(np_pt)
    return {"qt": qt, "kt": kt, "vt": vt, "wqk": wqk, "wv": wv, "wo": wo,
            "cmask": cm}


_NC_CACHE = {}


def _get_nc(key=("f32",), **kw):
    if key not in _NC_CACHE:
        _NC_CACHE[key] = build_nc(**kw)
    return _NC_CACHE[key]


# Matmul/storage configuration used by kernel().  "f32" is the fully
# fp32 (slow, reference-accurate) configuration; flipped after HW
# validation of the faster flavors.
import ml_dtypes

KCFG = {"mm": "bf16", "dt_x": BF16, "dt_pt": BF16, "dt_acc": BF16,
        "np_x": ml_dtypes.bfloat16, "np_pt": ml_dtypes.bfloat16}


def kernel(Q, K, V, w_q, w_k, w_v, w_o):
    Q = np.asarray(Q, dtype=np.float32)
    K = np.asarray(K, dtype=np.float32)
    V = np.asarray(V, dtype=np.float32)
    w_q = np.asarray(w_q, dtype=np.float32)
    w_k = np.asarray(w_k, dtype=np.float32)
    w_v = np.asarray(w_v, dtype=np.float32)
    w_o = np.asarray(w_o, dtype=np.float32)

    nc = _get_nc(key=(KCFG["mm"], str(KCFG["dt_x"])),
                 n=N_TOKENS, mm=KCFG["mm"], dt_x=KCFG["dt_x"],
                 dt_pt=KCFG["dt_pt"], dt_acc=KCFG["dt_acc"])
    in_maps = [
        prep_core_inputs(Q, K, V, w_q, w_k, w_v, w_o, c,
                         np_x=KCFG["np_x"], np_pt=KCFG["np_pt"])
        for c in range(N_CORES)
    ]
    res = bass_utils.run_bass_kernel_spmd(nc, in_maps,
                                          core_ids=list(range(N_CORES)))
    out = np.zeros((B, N_TOKENS, D_MODEL), dtype=np.float32)
    for c in range(N_CORES):
        out[c // 4] += res.results[c]["y"]
    return out


# revision 12
# speedup vs baseline: 1.2709x; 1.2709x over previous
"""Trainium2 Bass kernel for causal MHA (b=2, n=4096, d_model=768, 12 heads).

Sharding: 8 cores = 2 batches x 4 head-groups (3 heads each).
Each core:
  - receives its batch's Q/K/V pre-transposed ([768, n], d_model on rows)
    plus its head-group's weight slices (also pre-transposed on host).
  - projects qT/kT ([64, n] per head, head dim on partitions) and
    v ([n, 64] per head, tokens on partitions) on-chip.
  - computes scoresT[k, q] = kT^T @ qT tile-by-tile (128 keys x <=512
    queries, skipping the fully-masked left part of diagonal tiles),
    exponentiates (no max-subtraction: scores ~ N(0,1), fp32 exp is safe),
    masks the causal boundary block with a precomputed 0/1 mask, and
    accumulates outT_aug[65, q] += [v | ones]^T @ P in PSUM.  Row 64 is
    the softmax denominator; division is folded into the PSUM->SBUF copy.
  - applies the output projection with its w_o row-slice; host sums the
    4 partial outputs per batch (row-parallel linear unshard).

Weight-column host layout packs the six 64-wide q/k heads into three full
128-row M-blocks ([q0;q1], [q2;k2], [k0;k1]); k2 is then DMA-copied to
partitions 0-63 of a fourth block so every head's scores matmul sees its
qT and kT at the same partition base (a matmul constraint).  The same
DMA partition-shift turns outT into [h0;h1] + [h2] so the output
projection contracts in 2 chunks instead of 3.
"""

import sys

for _p in ("/opt/trn_rl_repo",):
    if _p not in sys.path:
        sys.path.insert(0, _p)

import numpy as np
import ml_dtypes

import concourse.bass as bass  # noqa: F401  (registers engine classes)
import concourse.tile as tile
from concourse import bacc, mybir
import concourse.bass_utils as bass_utils

P = 128
D_MODEL = 768
KO = D_MODEL // P  # 6 contraction chunks of 128
N_HEADS = 12
D_K = 64
N_CORES = 8
H_LOCAL = 3  # heads per core
D_LOCAL = H_LOCAL * D_K  # 192
B = 2
N_TOKENS = 4096
NQ = 512  # query-chunk size (one PSUM bank of fp32)
NT = 256  # token chunk for q/k projection

F32 = mybir.dt.float32
BF16 = mybir.dt.bfloat16
F32R = mybir.dt.float32r


def _mm(ap, flavor):
    """View an fp32 AP as the matmul input dtype."""
    if flavor == "f32r":
        return ap.bitcast(F32R)
    return ap


def build_nc(n=N_TOKENS, mm="bf16", dt_x=BF16, dt_pt=BF16, dt_acc=BF16):
    assert n % NQ == 0 and n % NT == 0 and n % P == 0
    nc = bacc.Bacc("TRN2", target_bir_lowering=False, debug=False,
                   num_devices=N_CORES)

    qt_d = nc.dram_tensor("qt", [D_MODEL, n], dt_x, kind="ExternalInput")
    kt_d = nc.dram_tensor("kt", [D_MODEL, n], dt_x, kind="ExternalInput")
    vt_d = nc.dram_tensor("vt", [D_MODEL, n], dt_x, kind="ExternalInput")
    wqk_d = nc.dram_tensor("wqk", [D_MODEL, 2 * D_LOCAL], dt_x,
                           kind="ExternalInput")
    wv_d = nc.dram_tensor("wv", [D_MODEL, D_LOCAL], dt_x, kind="ExternalInput")
    wo_d = nc.dram_tensor("wo", [D_LOCAL, D_MODEL], dt_x, kind="ExternalInput")
    cm_d = nc.dram_tensor("cmask", [P, P], dt_pt, kind="ExternalInput")
    y_d = nc.dram_tensor("y", [n, D_MODEL], F32, kind="ExternalOutput")

    qt_r = qt_d.ap().rearrange("(ko ki) t -> ki ko t", ki=P)
    kt_r = kt_d.ap().rearrange("(ko ki) t -> ki ko t", ki=P)
    vt_r = vt_d.ap().rearrange("(ko ki) t -> ki ko t", ki=P)
    wqk_r = wqk_d.ap().rearrange("(ko ki) m -> ki ko m", ki=P)
    wv_r = wv_d.ap().rearrange("(ko ki) m -> ki ko m", ki=P)

    TCH = n // NT       # q/k projection token chunks
    TB = n // P         # 128-token blocks
    QCH = n // NQ       # query chunks
    KB_PER_Q = NQ // P  # key blocks per query chunk (4)

    # Host weight-column order: [q0 q1 | q2 k2 | k0 k1] -> 3 full M-blocks.
    # qkT_sb blk3[0:64] is a DMA-shifted copy of k2 (blk1[64:128]).
    q_loc = {0: (0, 0), 1: (64, 0), 2: (0, 1)}
    k_loc = {0: (0, 2), 1: (64, 2), 2: (0, 3)}

    with tile.TileContext(nc) as tc:
        with tc.tile_pool(name="const", bufs=1) as cpool, \
             tc.tile_pool(name="persist", bufs=1) as ppool, \
             tc.tile_pool(name="xqk", bufs=3) as xpool, \
             tc.tile_pool(name="xv", bufs=2) as xvpool, \
             tc.tile_pool(name="pt", bufs=6) as ptpool, \
             tc.tile_pool(name="ysb", bufs=2) as ypool, \
             tc.tile_pool(name="rcp", bufs=2) as rpool, \
             tc.tile_pool(name="ot", bufs=2) as otpool, \
             tc.tile_pool(name="dbounce", bufs=2, space="DRAM") as dpool, \
             tc.tile_pool(name="pp_proj", bufs=2, space="PSUM") as pp_proj, \
             tc.tile_pool(name="pp_sc", bufs=3, space="PSUM") as pp_sc, \
             tc.tile_pool(name="pp_out", bufs=1, space="PSUM") as pp_out:

            # ---- constants ----
            wqk_sb = cpool.tile([P, KO, 2 * D_LOCAL], dt_x)
            nc.sync.dma_start(wqk_sb[:], wqk_r)
            wv_sb = cpool.tile([P, KO, D_LOCAL], dt_x)
            nc.sync.dma_start(wv_sb[:], wv_r)
            # w_o rows: chunk0 = dims of h0,h1 (128 rows), chunk1 = h2 (64)
            wo_sb = cpool.tile([P, 2, D_MODEL], dt_x)
            nc.sync.dma_start(wo_sb[:, 0, :], wo_d.ap()[0:P, :])
            nc.sync.dma_start(wo_sb[0:64, 1, :], wo_d.ap()[P:D_LOCAL, :])
            cm_sb = cpool.tile([P, P], dt_pt)
            nc.sync.dma_start(cm_sb[:], cm_d.ap())

            # ---- persistent activations ----
            qkT_sb = ppool.tile([P, 4, n], dt_acc)
            v_sb = ppool.tile([P, TB, H_LOCAL, 66], dt_acc)
            outT_sb = ppool.tile([P, 2, n], dt_acc)
            nc.vector.memset(v_sb[:, :, :, 64:65], 1.0)

            # ---- q/k projections (transposed layout, 3 packed M-blocks) ----
            for t in range(TCH):
                xq = xpool.tile([P, KO, NT], dt_x, tag="x")
                nc.sync.dma_start(xq[:], qt_r[:, :, t * NT:(t + 1) * NT])
                xk = xpool.tile([P, KO, NT], dt_x, tag="x")
                nc.sync.dma_start(xk[:], kt_r[:, :, t * NT:(t + 1) * NT])
                for blk in range(3):
                    ps = pp_proj.tile([P, NQ], F32, tag="psproj")
                    for ko in range(KO):
                        # blk1 contracts q2 against Q-input and k2 against
                        # K-input: split into two half-partition matmuls.
                        if blk == 1:
                            nc.tensor.matmul(
                                ps[0:64, 0:NT],
                                _mm(wqk_sb[:, ko, 128:192], mm),
                                _mm(xq[:, ko, :], mm),
                                start=(ko == 0), stop=(ko == KO - 1),
                                skip_group_check=True,
                            )
                            nc.tensor.matmul(
                                ps[64:128, 0:NT],
                                _mm(wqk_sb[:, ko, 192:256], mm),
                                _mm(xk[:, ko, :], mm),
                                start=(ko == 0), stop=(ko == KO - 1),
                                skip_group_check=True,
                            )
                        else:
                            x = xq if blk == 0 else xk
                            nc.tensor.matmul(
                                ps[:, 0:NT],
                                _mm(wqk_sb[:, ko, blk * 128:(blk + 1) * 128], mm),
                                _mm(x[:, ko, :], mm),
                                start=(ko == 0), stop=(ko == KO - 1),
                            )
                    nc.vector.tensor_copy(
                        out=qkT_sb[:, blk, t * NT:(t + 1) * NT],
                        in_=ps[:, 0:NT],
                    )
                # Partition-shifted copies so h2's scores matmul sees qT/kT
                # at the same base — and at BOTH bases, so h2 can alternate
                # row-groups and pair with whichever half is free:
                #   blk3[0:64]   = k2 (from blk1[64:128])
                #   blk3[64:128] = q2 (from blk1[0:64])
                nc.sync.dma_start(
                    qkT_sb[0:64, 3, t * NT:(t + 1) * NT],
                    qkT_sb[64:128, 1, t * NT:(t + 1) * NT],
                )
                nc.sync.dma_start(
                    qkT_sb[64:128, 3, t * NT:(t + 1) * NT],
                    qkT_sb[0:64, 1, t * NT:(t + 1) * NT],
                )

            # ---- v projection (token-major layout) ----
            for tb in range(TB):
                xv = xvpool.tile([P, KO, P], dt_x)
                nc.sync.dma_start(xv[:], vt_r[:, :, tb * P:(tb + 1) * P])
                ps = pp_proj.tile([P, NQ], F32, tag="psproj")
                for ko in range(KO):
                    nc.tensor.matmul(
                        ps[:, 0:D_LOCAL],
                        _mm(xv[:, ko, :], mm),
                        _mm(wv_sb[:, ko, :], mm),
                        start=(ko == 0), stop=(ko == KO - 1),
                    )
                for h in range(H_LOCAL):
                    nc.vector.tensor_copy(
                        out=v_sb[:, tb, h, 0:64],
                        in_=ps[:, h * 64:(h + 1) * 64],
                    )

            # ---- causal attention, transposed-score flash style ----
            # Heads are interleaved so the PE runs two concurrent score
            # matmuls on disjoint row-groups: h0 lives at partitions 0-63,
            # h1 at 64-127, h2 alternates base per key-block (its qT/kT are
            # replicated at both bases in blk1/blk3).
            def h2_qk(kb):
                if kb % 2 == 0:
                    return (0, 1), (0, 3)   # q2 @ blk1[0:64], k2' @ blk3[0:64]
                return (64, 3), (64, 1)     # q2' @ blk3[64:128], k2 @ blk1[64:128]

            def qk_for(h, kb):
                if h == 2:
                    return h2_qk(kb)
                return q_loc[h], k_loc[h]

            for j in range(QCH):
                po = [pp_out.tile([P, NQ], F32, tag=f"po{h}", name=f"po{h}")
                      for h in range(H_LOCAL)]
                nkb = KB_PER_Q * j + KB_PER_Q
                for kb2 in range(0, nkb, 2):
                    # pairing order: [s_h0||s_h1](kb2), [s_h2(kb2)||s_h2(kb2+1)],
                    # [s_h0||s_h1](kb2+1)
                    order = [(0, kb2), (1, kb2), (2, kb2), (2, kb2 + 1),
                             (0, kb2 + 1), (1, kb2 + 1)]
                    pts = {}
                    for (h, kb) in order:
                        (qp, qb), (kp, kb_) = qk_for(h, kb)
                        kloc = kb - KB_PER_Q * j
                        off = max(kloc, 0) * P  # masked part of diag tiles
                        psc = pp_sc.tile([P, NQ], F32, tag="psc", name="psc")
                        nc.tensor.matmul(
                            psc[:, off:],
                            _mm(qkT_sb[kp:kp + 64, kb_, kb * P:(kb + 1) * P], mm),
                            _mm(qkT_sb[qp:qp + 64, qb,
                                       j * NQ + off:(j + 1) * NQ], mm),
                            start=True, stop=True,
                        )
                        pt = ptpool.tile([P, NQ], dt_pt, name="pt")
                        nc.scalar.activation(pt[:, off:], psc[:, off:],
                                             mybir.ActivationFunctionType.Exp)
                        if kloc >= 0:
                            nc.vector.tensor_mul(out=pt[:, off:off + P],
                                                 in0=pt[:, off:off + P],
                                                 in1=cm_sb[:])
                        pts[(h, kb)] = (pt, off)
                    for (h, kb) in order:
                        pt, off = pts[(h, kb)]
                        nc.tensor.matmul(
                            po[h][0:65, off:],
                            _mm(v_sb[:, kb, h, 0:65], mm),
                            _mm(pt[:, off:], mm),
                            start=(kb == 0), stop=(kb == nkb - 1),
                        )
                # Copy each po to SBUF right away (frees the PSUM bank for
                # the next chunk), then normalize from SBUF.
                for h in range(H_LOCAL):
                    raw = rpool.tile([65, NQ], F32, tag="raw", name="raw")
                    nc.vector.tensor_copy(out=raw[:], in_=po[h][0:65, :])
                    r1 = rpool.tile([65, NQ], F32, tag="r1", name="r1")
                    nc.vector.reciprocal(r1[64:65, :], raw[64:65, :])
                    db = dpool.tile([1, NQ], F32, name="db")
                    nc.sync.dma_start(db[:], r1[64:65, :])
                    rr = rpool.tile([64, NQ], F32, tag="rr", name="rr")
                    nc.sync.dma_start(rr[:], db[:].to_broadcast((64, NQ)))
                    if h == 1:
                        # h1 lives at partitions 64-127 of outT blk0; DVE
                        # lanes are partition-locked, so write a temp at
                        # base 0 and DMA partition-shift it up.
                        ot = otpool.tile([64, NQ], dt_acc, name="ot")
                        nc.vector.tensor_mul(out=ot[:], in0=raw[0:64, :],
                                             in1=rr[:])
                        nc.sync.dma_start(
                            outT_sb[64:128, 0, j * NQ:(j + 1) * NQ], ot[:])
                    else:
                        dst = outT_sb[0:64, 0 if h == 0 else 1,
                                      j * NQ:(j + 1) * NQ]
                        nc.vector.tensor_mul(out=dst, in0=raw[0:64, :],
                                             in1=rr[:])

            # ---- output projection (K = 128 + 64) ----
            NOC = 2  # 768 = 2 x 384
            NO = D_MODEL // NOC
            for tb in range(TB):
                for oc in range(NOC):
                    ps = pp_proj.tile([P, NQ], F32, tag="psproj")
                    nc.tensor.matmul(
                        ps[:, 0:NO],
                        _mm(outT_sb[:, 0, tb * P:(tb + 1) * P], mm),
                        _mm(wo_sb[:, 0, oc * NO:(oc + 1) * NO], mm),
                        start=True, stop=False,
                    )
                    nc.tensor.matmul(
                        ps[:, 0:NO],
                        _mm(outT_sb[0:64, 1, tb * P:(tb + 1) * P], mm),
                        _mm(wo_sb[0:64, 1, oc * NO:(oc + 1) * NO], mm),
                        start=False, stop=True,
                    )
                    ysb = ypool.tile([P, NO], F32)
                    nc.vector.tensor_copy(out=ysb[:], in_=ps[:, 0:NO])
                    nc.sync.dma_start(
                        y_d.ap()[tb * P:(tb + 1) * P, oc * NO:(oc + 1) * NO],
                        ysb[:],
                    )

    nc.compile()
    return nc


def make_causal_mask_np(dt=np.float32):
    """[128, 128] lower-left keep mask: m[p, f] = 1.0 iff f >= p."""
    f = np.arange(P)[None, :]
    p = np.arange(P)[:, None]
    return (f >= p).astype(np.float32).astype(dt)


def prep_core_inputs(Q, K, V, w_q, w_k, w_v, w_o, core, n=N_TOKENS,
                     np_x=ml_dtypes.bfloat16, np_pt=ml_dtypes.bfloat16):
    """Host-side sharding/layout prep for one core. All fp32 numpy in."""
    b = core // 4
    g = core % 4
    hs = g * D_LOCAL
    scale = 1.0 / np.sqrt(D_K)
    qt = np.ascontiguousarray(Q[b].T).astype(np_x)
    kt = np.ascontiguousarray(K[b].T).astype(np_x)
    vt = np.ascontiguousarray(V[b].T).astype(np_x)
    wql = w_q[hs:hs + D_LOCAL] * scale
    wkl = w_k[hs:hs + D_LOCAL]
    # column order [q0 q1 | q2 k2 | k0 k1] (see build_nc)
    wqk = np.ascontiguousarray(
        np.concatenate([wql[0:128], wql[128:192], wkl[128:192], wkl[0:128]],
                       axis=0).T
    ).astype(np_x)
    wv = np.ascontiguousarray(w_v[hs:hs + D_LOCAL].T).astype(np_x)
    wo = np.ascontiguousarray(w_o[:, hs:hs + D_LOCAL].T).astype(np_x)
    cm = make_causal_mask_np(np_pt)
    return {"qt": qt, "kt": kt, "vt": vt, "wqk": wqk, "wv": wv, "wo": wo,
            "cmask": cm}


_NC_CACHE = {}


def _get_nc(key, **kw):
    if key not in _NC_CACHE:
        _NC_CACHE[key] = build_nc(**kw)
    return _NC_CACHE[key]


KCFG = {"mm": "bf16", "dt_x": BF16, "dt_pt": BF16, "dt_acc": BF16,
        "np_x": ml_dtypes.bfloat16, "np_pt": ml_dtypes.bfloat16}


def kernel(Q, K, V, w_q, w_k, w_v, w_o):
    Q = np.asarray(Q, dtype=np.float32)
    K = np.asarray(K, dtype=np.float32)
    V = np.asarray(V, dtype=np.float32)
    w_q = np.asarray(w_q, dtype=np.float32)
    w_k = np.asarray(w_k, dtype=np.float32)
    w_v = np.asarray(w_v, dtype=np.float32)
    w_o = np.asarray(w_o, dtype=np.float32)

    nc = _get_nc((KCFG["mm"], str(KCFG["dt_x"])),
                 n=N_TOKENS, mm=KCFG["mm"], dt_x=KCFG["dt_x"],
                 dt_pt=KCFG["dt_pt"], dt_acc=KCFG["dt_acc"])
    in_maps = [
        prep_core_inputs(Q, K, V, w_q, w_k, w_v, w_o, c,
                         np_x=KCFG["np_x"], np_pt=KCFG["np_pt"])
        for c in range(N_CORES)
    ]
    res = bass_utils.run_bass_kernel_spmd(nc, in_maps,
                                          core_ids=list(range(N_CORES)))
    out = np.zeros((B, N_TOKENS, D_MODEL), dtype=np.float32)
    for c in range(N_CORES):
        out[c // 4] += res.results[c]["y"]
    return out
